# revision 36
# baseline (speedup 1.0000x reference)
"""Trainium2 Bass kernel for nn_AaD_MAPU (retrieval kNN + KL attraction / dispersion loss).

Reference computation:
    softmax_out = softmax(predictions)                      [B,C]
    f_norm      = l2_normalize(features)                    [B,D]
    fb          = fea_bank with rows trg_idx <- f_norm      [N,D]
    sb          = score_bank with rows trg_idx <- softmax   [N,C]
    distance    = f_norm @ fb.T                             [B,N]
    idx         = top_k(distance, K+1); idx_near = idx[:,1:]
    score_near  = sb[idx_near]                              [B,K,C]
    loss        = sum(score_near * (log(score_near) - softmax[:,None,:])) / B
    neg_pred    = mean(rowsum(softmax @ softmax.T - diag))
    out         = loss + neg_pred

Device strategy (8 NeuronCores, bank rows sharded, d-major layout):
  - Pad bank to 100352 rows; core c owns rows [c*12544, (c+1)*12544).
  - The host ships each shard transposed ([D, 12544]) and fp8-e4m3 cast
    (f_norm side pre-scaled x16 so unit-norm entries stay well above the
    fp8 subnormal range; a uniform scale cannot change the ranking).
  - fp8 DoubleRow matmuls (2 d-chunks per instruction, 0.5 cyc/row)
    accumulate PSUM fp32 [128b, 512j] distance tiles.
  - ScalarE copies each PSUM tile to SBUF bf16; VectorE keeps a running
    elementwise max ("comb max") over the j-tiles in bf16 2x mode:
    comb[b, e] = max_t distance[b, t*512 + e], e in [0, 512).
  - VectorE max8 + find_index8 per 128-row chunk -> top-8 (comb max, e).
Host merges 8 cores x 8 candidate combs per row, recomputes each winning
comb's ~25 member distances in exact fp32 (so fp8/bf16 noise cannot affect
the final selection), takes the top-2 per comb (covers two neighbours
landing in one comb), re-ranks, drops the top-1 (the reference drops
idx[:,0]), gathers scores and reduces the loss.
"""

from contextlib import ExitStack

import numpy as np

import concourse.bass as bass
import concourse.tile as tile
from concourse import bacc, mybir
from concourse.bass_utils import run_bass_kernel_spmd

# Problem constants (hardcoded per contest rules).
B, D, N, C, K = 512, 512, 100000, 64, 5
EPS = 1e-12
NCORES = 8
NSHARD = 12544            # padded bank rows per core (98 * 128)
NPAD = NSHARD * NCORES    # 100352
BCH = 4                   # batch chunks of 128 rows
JT = 512                  # j-tile width == comb count per core
NJT = 25                  # 24 full tiles + one 256-wide tile
N_WARMUP = 8              # zero matmuls to warm the PE during the first DMA
FSCALE = 16.0             # f_norm pre-scale so fp8 quantization is well-conditioned

_F32 = mybir.dt.float32
_BF16 = mybir.dt.bfloat16
_FP8 = mybir.dt.float8e4
_U32 = mybir.dt.uint32

_cache = {}


def _build_module():
    nc = bacc.Bacc("TRN2", target_bir_lowering=False, debug=False,
                   num_devices=NCORES)
    # bank shard, transposed + fp8-cast on host: [D, NSHARD]
    fbt_d = nc.dram_tensor("fbt", [D, NSHARD], _FP8, kind="ExternalInput").ap()
    # f_norm.T (pre-scaled, fp8) packed on host as [dp, dc*B + b]
    fnt_d = nc.dram_tensor("fnt", [128, 4 * B], _FP8, kind="ExternalInput").ap()
    # top-8 combs per 128-row chunk: [..., 0, :] = value (fp32 bits),
    # [..., 1, :] = comb id — one tensor so the tail is a single tiny DMA
    cat_out = nc.dram_tensor("cat_out", [128, BCH, 2, 8], _U32,
                             kind="ExternalOutput").ap()

    with tile.TileContext(nc) as tc, ExitStack() as ctx:
        const = ctx.enter_context(tc.tile_pool(name="const", bufs=1))
        fbt_pool = ctx.enter_context(tc.tile_pool(name="fbt", bufs=4))
        dp_pool = ctx.enter_context(tc.tile_pool(name="dp", bufs=2, space="PSUM"))
        tmp_pool = ctx.enter_context(tc.tile_pool(name="tmp", bufs=4))
        out_pool = ctx.enter_context(tc.tile_pool(name="outs", bufs=1))

        # PE warm-up: harmless zero matmuls that run while the first DMAs land
        wu_sb = const.tile([128, JT], _F32)
        nc.gpsimd.memset(wu_sb[:], 0.0)
        wu_ps = dp_pool.tile([128, BCH, JT], _F32, tag="dp")
        wu_r = wu_sb[:].bitcast(_FP8).rearrange("p (c j) -> p c j", c=4)
        for _ in range(N_WARMUP):
            nc.tensor.matmul(wu_ps[:, 0], lhsT=wu_r[:, 0:2, :128], rhs=wu_r[:, 0:2],
                             start=True, stop=True,
                             perf_mode=mybir.MatmulPerfMode.DoubleRow)

        fnt_sb = const.tile([128, 4, B], _FP8)
        nc.sync.dma_start(fnt_sb[:], fnt_d.rearrange("p (c b) -> p c b", c=4))

        # running comb maxima, bf16; two parity-striped accumulators so the
        # ScalarE copy -> VectorE fold chains of consecutive tiles pipeline
        acc = [const.tile([128, BCH, JT], _BF16, name=f"acc{i}") for i in range(2)]
        cat = out_pool.tile([128, BCH, 2, 8], _U32)

        for t in range(NJT):
            j0 = t * JT
            W = min(JT, NSHARD - j0)

            # bank tile in [d, j] layout: partition = d % 128, c = d // 128
            fbt = fbt_pool.tile([128, 4, JT], _FP8, tag="fbt")
            nc.sync.dma_start(
                fbt[:, :, :W],
                fbt_d[:, j0:j0 + W].rearrange("(c p) j -> p c j", p=128),
            )

            dp = dp_pool.tile([128, BCH, JT], _F32, tag="dp")
            for bc in range(BCH):
                for h in range(2):        # DoubleRow: two d-chunks per matmul
                    nc.tensor.matmul(
                        dp[:, bc, :W],
                        lhsT=fnt_sb[:, 2 * h:2 * h + 2, bc * 128:(bc + 1) * 128],
                        rhs=fbt[:, 2 * h:2 * h + 2, :W],
                        start=(h == 0), stop=(h == 1),
                        perf_mode=mybir.MatmulPerfMode.DoubleRow,
                    )
            a = acc[t % 2][:, :, :W]
            if t < 2:
                nc.scalar.copy(out=a, in_=dp[:, :, :W])
            elif t % 5 == 2:
                # a few tiles fold straight from PSUM on the VectorE to keep
                # the ScalarE (copy) and VectorE (fold) loads balanced
                nc.vector.tensor_max(a, a, dp[:, :, :W])
            else:
                tmp = tmp_pool.tile([128, BCH, JT], _BF16, tag="tmp")
                nc.scalar.copy(out=tmp[:, :, :W], in_=dp[:, :, :W])
                nc.vector.tensor_max(a, a, tmp[:, :, :W])

        nc.vector.tensor_max(acc[0][:], acc[0][:], acc[1][:])
        for bc in range(BCH):
            sl = acc[0][:, bc]
            v8 = cat[:, bc, 0].bitcast(_F32)
            nc.vector.max(out=v8, in_=sl)
            nc.vector.max_index(out=cat[:, bc, 1], in_max=v8, in_values=sl)
        nc.sync.dma_start(cat_out, cat[:])

    nc.compile()
    return nc


def _get_module():
    if "nc" not in _cache:
        _cache["nc"] = _build_module()
    return _cache["nc"]


def kernel(features, predictions, fea_bank, score_bank, trg_idx):
    features = np.asarray(features, dtype=np.float32)
    predictions = np.asarray(predictions, dtype=np.float32)
    fea_bank = np.asarray(fea_bank, dtype=np.float32)
    score_bank = np.asarray(score_bank, dtype=np.float32)
    trg_idx = np.asarray(trg_idx, dtype=np.int32)

    # ---- tiny host prologue (O(B*D)) ----
    sm = predictions - predictions.max(axis=1, keepdims=True)
    np.exp(sm, out=sm)
    sm /= sm.sum(axis=1, keepdims=True)                       # softmax_out [B,C]
    nrm = np.maximum(np.sqrt((features * features).sum(axis=1, keepdims=True)),
                     EPS)
    f_norm = features / nrm                                   # [B,D]

    # bank updates + padding
    fbp = np.zeros((NPAD, D), dtype=np.float32)
    fbp[:N] = fea_bank
    fbp[trg_idx] = f_norm
    sb = score_bank.copy()
    sb[trg_idx] = sm

    # f_norm.T (pre-scaled for fp8 conditioning) packed as [dp, dc*B + b]
    import ml_dtypes
    fp8 = ml_dtypes.float8_e4m3
    fnt = np.ascontiguousarray(
        (f_norm.T * FSCALE).reshape(4, 128, B).transpose(1, 0, 2)
        .reshape(128, 4 * B)).astype(fp8)

    nc = _get_module()
    in_maps = [
        {"fbt": np.ascontiguousarray(
             fbp[c * NSHARD:(c + 1) * NSHARD].T).astype(fp8),
         "fnt": fnt}
        for c in range(NCORES)
    ]
    res = run_bass_kernel_spmd(nc, in_maps, core_ids=list(range(NCORES)))

    # ---- host epilogue: merge candidate combs, resolve in fp32, loss ----
    # outputs are [128, BCH, 8]; row b = bc*128 + p
    vals = np.empty((B, NCORES * 8), np.float32)
    core = np.empty((B, NCORES * 8), np.int64)
    comb = np.empty((B, NCORES * 8), np.int64)
    for c, r in enumerate(res.results):
        cat = np.ascontiguousarray(r["cat_out"].transpose(1, 0, 2, 3))
        vals[:, c * 8:(c + 1) * 8] = cat[:, :, 0].view(np.float32).reshape(B, 8)
        comb[:, c * 8:(c + 1) * 8] = cat[:, :, 1].astype(np.int64).reshape(B, 8)
        core[:, c * 8:(c + 1) * 8] = c

    # preselect by the (fp8/bf16-precision) device values, then re-rank the
    # short list with exact fp32 dots so quantization noise cannot leak in
    TOP = 12
    order = np.argsort(-vals, axis=1, kind="stable")[:, :TOP]
    top_core = np.take_along_axis(core, order, axis=1)        # [B, TOP]
    top_comb = np.take_along_axis(comb, order, axis=1)        # [B, TOP]

    tt = np.arange(NJT, dtype=np.int64)[None, None, :]
    pos_local = top_comb[:, :, None] + tt * JT                # [B, TOP, NJT]
    valid = pos_local < NSHARD
    rows = top_core[:, :, None] * NSHARD + np.minimum(pos_local, NSHARD - 1)
    vecs = fbp[rows.reshape(-1)].reshape(B, TOP, NJT, D)
    dots = np.einsum("rktd,rd->rkt", vecs, f_norm, optimize=True)
    dots = np.where(valid & (rows < N), dots, np.float32(-np.inf))

    # top-2 member rows per comb (two neighbours may share one comb)
    p2 = np.argsort(-dots, axis=2)[:, :, :2]                  # [B, TOP, 2]
    v2 = np.take_along_axis(dots, p2, axis=2).reshape(B, 2 * TOP)
    i2 = np.take_along_axis(rows, p2, axis=2).reshape(B, 2 * TOP)

    # order exactly like jax.lax.top_k: value desc, index asc on ties
    reorder = np.lexsort((i2, -v2), axis=1)
    top_idx = np.take_along_axis(i2, reorder, axis=1)

    idx_near = top_idx[:, 1:K + 1]                            # drop self slot 0
    score_near = sb[idx_near].astype(np.float64)              # [B,K,C]
    kl = score_near * (np.log(score_near) - sm[:, None, :].astype(np.float64))
    loss = kl.sum(axis=(1, 2)).mean()

    s64 = sm.astype(np.float64)
    neg_pred = (np.square(s64.sum(axis=0)).sum()
                - np.square(s64).sum()) / B

    return np.float32(loss + neg_pred)
